# revision 6
# baseline (speedup 1.0000x reference)
"""Causal multi-head attention block (B=2, L=2048, D=1024, H=16) on 8 trn2 cores.

Sharding: core c -> batch b = c // 4, head group g = c % 4 (heads 4g..4g+4).
All matmul operands bf16 (fp32 PSUM accumulation); rel-err gate is 2e-2.

Per core:
  1. QT/KT = (W_qk x^T + b)      (d_head on partitions; 512 x 2048)
  2. V     = (x W_v^T + b_v)     (token j on partitions; [128, 256] per j-chunk)
  3. per (query-block t_, head-pair hp), J = 0..4(t_+1)-1:
     - ST for both heads into one [128,1024] PSUM tile (row-tiled: head A uses
       PE rows 0-63, head B rows 64-127 -> concurrent), causal-narrowed on the
       diagonal; PSUM groups there opened by a bf16 mask matmul
       (sum_k u[k,jr] w[k,ic] = -1e30*max(0, jr-ic)).
     - one exp (no-max softmax, scores ~N(0,1)) -> bf16 P tile
     - PV col-tiled: head A -> PSUM partitions 0-63, head B -> 64-127
       (concurrent); denominators via 2-way col-tiled ones-matmuls into a
       shared per-t_ PSUM tile at partitions 0/32/64/96.
  4. per t_: one reciprocal (PSUM den -> bf16), sel-matmul broadcast,
     elementwise scale -> bf16 ot_t; y^T_partial = W_out,local ot_t -> bf16 DMA
Host: y[b] = sum of the 4 partials^T + b_out.
QKV / out-projection matmuls are interleaved between attention units to keep
the PE busy (HAM stays at full clock).
"""

import numpy as np
import ml_dtypes

import concourse.bass as bass
import concourse.bacc as bacc
import concourse.mybir as mybir
from concourse.tile import TileContext
from concourse.bass_utils import run_bass_kernel_spmd

B, L, D, H = 2, 2048, 1024, 16
HD = 64                      # head dim
HPC = 4                      # heads per core
DL = HPC * HD                # 256 local head dims
N_CORES = 8
NEG = -1.0e30
SCALE = 1.0 / 8.0            # 1/sqrt(64)
FP32 = mybir.dt.float32
BF16 = mybir.dt.bfloat16
AF = mybir.ActivationFunctionType
BF = ml_dtypes.bfloat16

NKC = D // 128               # 8 contraction chunks over D
NMB = L // 512               # 4 column blocks of 512 over L
NJC = L // 128               # 16 j-chunks of 128


def build_program():
    nc = bacc.Bacc("TRN2", target_bir_lowering=False, debug=False)

    xt = nc.dram_tensor("xt", [D, L], BF16, kind="ExternalInput")
    wqk = nc.dram_tensor("wqk", [D, 2 * DL], BF16, kind="ExternalInput")
    wv = nc.dram_tensor("wv", [D, DL], BF16, kind="ExternalInput")
    wout = nc.dram_tensor("wout", [DL, D], BF16, kind="ExternalInput")
    bqk = nc.dram_tensor("bqk", [2 * DL, 1], FP32, kind="ExternalInput")
    bv = nc.dram_tensor("bv", [1, DL], FP32, kind="ExternalInput")
    umd = nc.dram_tensor("umd", [128, 128], BF16, kind="ExternalInput")
    wmd = nc.dram_tensor("wmd", [128, 512], BF16, kind="ExternalInput")
    seld = nc.dram_tensor("seld", [128, 256], BF16, kind="ExternalInput")
    yt = nc.dram_tensor("yt", [D, L], BF16, kind="ExternalOutput")

    with TileContext(nc) as tc:
        with (
            tc.tile_pool(name="const", bufs=1) as const,
            tc.tile_pool(name="xtp", bufs=24) as xtp,
            tc.tile_pool(name="ptp", bufs=4) as ptp,
            tc.tile_pool(name="rp", bufs=2) as rp,
            tc.tile_pool(name="yp", bufs=2) as yp,
            tc.tile_pool(name="ps_st", bufs=2, space="PSUM") as ps_st,
            tc.tile_pool(name="ps_ot", bufs=1, space="PSUM") as ps_ot,
            tc.tile_pool(name="ps_den", bufs=1, space="PSUM") as ps_den,
            tc.tile_pool(name="ps_sm", bufs=2, space="PSUM") as ps_sm,
        ):
            # ---- persistent constants / weights ----
            # (DMA issue order matters: the first qk-unit needs wqk + bqk + x
            # block 0, so those are queued first.)
            wqk_t = []
            xts0 = []
            for kc in range(NKC):
                t = const.tile([128, 2 * DL], BF16, tag=f"wqk{kc}")
                nc.sync.dma_start(out=t[:], in_=wqk[kc * 128:(kc + 1) * 128, :])
                wqk_t.append(t)
                tx = xtp.tile([128, 512], BF16, name="t")
                nc.sync.dma_start(
                    out=tx[:], in_=xt[kc * 128:(kc + 1) * 128, 0:512])
                xts0.append(tx)
            bq_t = []
            for nt in range(4):
                t = const.tile([128, 1], FP32, tag=f"bqk{nt}")
                nc.sync.dma_start(out=t[:], in_=bqk[nt * 128:(nt + 1) * 128, :])
                bq_t.append(t)

            def load_consts2():
                wv_t = []
                for kc in range(NKC):
                    t = const.tile([128, DL], BF16, tag=f"wv{kc}")
                    nc.sync.dma_start(out=t[:],
                                      in_=wv[kc * 128:(kc + 1) * 128, :])
                    wv_t.append(t)
                bvrep = const.tile([128, DL], FP32, tag="bvrep")
                nc.sync.dma_start(out=bvrep[:],
                                  in_=bv[0:1, :].to_broadcast((128, DL)))
                um_t = const.tile([128, 128], BF16, tag="um")
                nc.sync.dma_start(out=um_t[:], in_=umd[:, :])
                wm_t = const.tile([128, 512], BF16, tag="wm")
                nc.sync.dma_start(out=wm_t[:], in_=wmd[:, :])
                sel_t = const.tile([128, 256], BF16, tag="sel")
                nc.sync.dma_start(out=sel_t[:], in_=seld[:, :])
                return wv_t, bvrep, um_t, wm_t, sel_t

            def load_consts3():
                wout_t = []
                for n2 in range(2):
                    t = const.tile([128, D], BF16, tag=f"wout{n2}")
                    nc.sync.dma_start(out=t[:],
                                      in_=wout[n2 * 128:(n2 + 1) * 128, :])
                    wout_t.append(t)
                return wout_t

            # persistent activations
            # qk_t[0..1]: QT tiles (128 rows: heads {2i,2i+1}); qk_t[2..3]: KT
            qk_t = [const.tile([128, L], BF16, tag=f"qk{nt}", name=f"qk{nt}")
                    for nt in range(4)]
            # V tiles per j-chunk: [128, 256]; head h cols h*64..h*64+64
            v_t = [const.tile([128, DL], BF16, tag=f"v{j}", name=f"v{j}")
                   for j in range(NJC)]
            ot_t = [const.tile([128, L], BF16, tag=f"ot{n2}", name=f"ot{n2}")
                    for n2 in range(2)]

            def load_x(m):
                xts = []
                for kc in range(NKC):
                    t = xtp.tile([128, 512], BF16)
                    nc.sync.dma_start(
                        out=t[:],
                        in_=xt[kc * 128:(kc + 1) * 128, m * 512:(m + 1) * 512])
                    xts.append(t)
                return xts

            def qk_unit(xts, m, nt):
                ps = ps_sm.tile([128, 512], FP32, tag="ps_sm")
                for kc in range(NKC):
                    nc.tensor.matmul(
                        ps[:],
                        wqk_t[kc][:, nt * 128:(nt + 1) * 128],
                        xts[kc][:],
                        start=(kc == 0), stop=(kc == NKC - 1))
                with nc.allow_low_precision(reason="bf16 activations"):
                    nc.vector.tensor_scalar_add(
                        qk_t[nt][:, m * 512:(m + 1) * 512], ps[:], bq_t[nt][:])

            def v_unit(xts, m, ic):
                j = 4 * m + ic
                ps = ps_sm.tile([128, 512], FP32, tag="ps_sm")
                for kc in range(NKC):
                    nc.tensor.matmul(
                        ps[:, 0:DL],
                        xts[kc][:, ic * 128:(ic + 1) * 128],
                        wv_t[kc][:],
                        start=(kc == 0), stop=(kc == NKC - 1))
                with nc.allow_low_precision(reason="bf16 activations"):
                    nc.vector.tensor_add(v_t[j][:], ps[:, 0:DL], bvrep[:])

            def attn_pair(hp, t_, den, filler):
                """One head pair's ST -> exp -> PV/den chain over all j-chunks,
                software-pipelined one unit ahead so the scalar engine's exps
                run back-to-back. ST row-tiled (heads at PE rows 0-63/64-127),
                PV col-tiled (output partitions 0-63/64-127) -> concurrent."""
                n_j = 4 * (t_ + 1)
                qt = qk_t[hp]
                kt = qk_t[2 + hp]
                otp = ps_ot.tile([128, 512], FP32, tag="ps_ot",
                                 name=f"otp{hp}")

                def do_st(J):
                    q = J - 4 * t_      # >= 0 on the diagonal band
                    w0 = 128 * q if q > 0 else 0
                    stp = ps_st.tile([128, 1024], FP32, tag="ps_st",
                                     name="stp")
                    for i in range(2):
                        po = i * 64
                        ssl = slice(i * 512 + w0, (i + 1) * 512)
                        if q >= 0:
                            # open the psum group with the causal mask:
                            # sum_k u[k,jr] w[k,ic] = -1e30*max(0, jr-ic)
                            nc.tensor.matmul(
                                stp[:, ssl],
                                um_t[:], wm_t[:, 0:512 - w0],
                                start=True, stop=False)
                        nc.tensor.matmul(
                            stp[:, ssl],
                            kt[po:po + 64, J * 128:(J + 1) * 128],
                            qt[po:po + 64, t_ * 512 + w0:(t_ + 1) * 512],
                            start=(q < 0), stop=True)
                    return stp

                stp_next = do_st(0)
                for J in range(n_j):
                    q = J - 4 * t_
                    w0 = 128 * q if q > 0 else 0
                    stp = stp_next
                    ptile = ptp.tile([128, 1024], BF16, name="pt")
                    if q >= 0:
                        for i in range(2):
                            nc.scalar.activation(
                                ptile[:, i * 512 + w0:(i + 1) * 512],
                                stp[:, i * 512 + w0:(i + 1) * 512],
                                AF.Exp, scale=SCALE)
                    else:
                        nc.scalar.activation(ptile[:], stp[:],
                                             AF.Exp, scale=SCALE)
                    if J + 1 < n_j:
                        stp_next = do_st(J + 1)
                    filler()
                    for i in range(2):
                        h = 2 * hp + i
                        nc.tensor.matmul(
                            otp[i * 64:(i + 1) * 64, w0:512],
                            v_t[J][:, (h % 4) * 64:(h % 4) * 64 + 64],
                            ptile[:, i * 512 + w0:(i + 1) * 512],
                            start=(J == 0), stop=(J == n_j - 1),
                            skip_group_check=True)
                    for i in range(2):
                        dr = 32 * (2 * hp + i)
                        nc.tensor.matmul(
                            den[dr:dr + 1, w0:512],
                            um_t[:, 127:128],
                            ptile[:, i * 512 + w0:(i + 1) * 512],
                            start=(J == 0), stop=(J == n_j - 1),
                            tile_position=(0, dr),
                            skip_group_check=True)
                # copy O^T|pair out of PSUM (releases the PV psum slot)
                osb = rp.tile([128, 512], FP32, name="osb", tag=f"osb{hp}")
                nc.vector.tensor_copy(osb[:], otp[:])
                return osb

            def norm_finish(osbs, den, t_):
                """One reciprocal for all 4 heads' denominators, broadcast via
                sel-matmul, scale into ot_t."""
                isl = slice(t_ * 512, (t_ + 1) * 512)
                linv = rp.tile([128, 512], BF16, name="linv", tag="linv")
                with nc.allow_low_precision(reason="bf16 norm scale"):
                    nc.vector.reciprocal(linv[:], den[:])
                for n2 in range(2):
                    rb = ps_sm.tile([128, 512], FP32, tag="ps_sm")
                    nc.tensor.matmul(rb[:],
                                     sel_t[:, n2 * 128:(n2 + 1) * 128],
                                     linv[:], start=True, stop=True)
                    with nc.allow_low_precision(reason="bf16 activations"):
                        nc.vector.tensor_mul(ot_t[n2][:, isl], osbs[n2][:],
                                             rb[:])

            def proj_unit(t_, dt_):
                isl = slice(t_ * 512, (t_ + 1) * 512)
                ps = ps_sm.tile([128, 512], FP32, tag="ps_sm")
                for n2 in range(2):
                    nc.tensor.matmul(
                        ps[:],
                        wout_t[n2][:, dt_ * 128:(dt_ + 1) * 128],
                        ot_t[n2][:, isl],
                        start=(n2 == 0), stop=(n2 == 1))
                ys = yp.tile([128, 512], BF16, name="ys")
                with nc.allow_low_precision(reason="bf16 output"):
                    nc.vector.tensor_copy(ys[:], ps[:])
                nc.sync.dma_start(
                    out=yt[dt_ * 128:(dt_ + 1) * 128, isl], in_=ys[:])

            # ---- program ----
            wv_t, bvrep, um_t, wm_t, sel_t = load_consts2()
            # only what t_=0 pair 0 needs up front: QT/KT heads 0-1 + first
            # two V chunks; the rest of block 0 becomes t_=0 filler work
            qk_unit(xts0, 0, 0)
            qk_unit(xts0, 0, 2)
            v_unit(xts0, 0, 0)
            v_unit(xts0, 0, 1)
            wout_t = load_consts3()

            # attention block order (0, 1, 3, 2): t_=0 starts right after
            # QKV block 0; its fillers compute QKV blocks 1-2; t_=1 gets
            # block 3 + proj(0); t_=3 gets proj(1); t_=2 gets proj(3);
            # proj(2) trails at the end.
            filler_plan = {0: [], 1: [], 3: [], 2: []}
            filler_plan[0].append(lambda: v_unit(xts0, 0, 2))
            filler_plan[0].append(lambda: v_unit(xts0, 0, 3))
            filler_plan[0].append(lambda: qk_unit(xts0, 0, 1))
            filler_plan[0].append(lambda: qk_unit(xts0, 0, 3))
            for m in (1, 2):
                xts = load_x(m)
                for u in range(4):
                    filler_plan[0].append(
                        lambda u=u, xts=xts, m=m: qk_unit(xts, m, u))
                    filler_plan[0].append(
                        lambda u=u, xts=xts, m=m: v_unit(xts, m, u))
            xts3 = load_x(3)
            for u in range(4):
                filler_plan[1].append(
                    lambda u=u: qk_unit(xts3, 3, u))
                filler_plan[1].append(
                    lambda u=u: v_unit(xts3, 3, u))
            for dt_ in range(8):
                filler_plan[1].append(lambda dt_=dt_: proj_unit(0, dt_))
                filler_plan[3].append(lambda dt_=dt_: proj_unit(1, dt_))
                filler_plan[2].append(lambda dt_=dt_: proj_unit(3, dt_))

            for t_ in (0, 1, 3, 2):
                units = filler_plan[t_]
                n_slots = 2 * 4 * (t_ + 1)   # filler call sites this block
                state = {"i": 0, "slot": 0}

                def filler(state=state, units=units, n_slots=n_slots):
                    # spread the unit supply evenly over the block's slots
                    state["slot"] += 1
                    want = (len(units) * state["slot"] + n_slots - 1) // n_slots
                    while state["i"] < min(want, len(units)):
                        units[state["i"]]()
                        state["i"] += 1

                den = ps_den.tile([128, 512], FP32, tag="ps_den",
                                  name=f"den{t_}")
                nc.vector.memset(den[:], 1.0)
                osbs = []
                for hp in range(2):
                    osbs.append(attn_pair(hp, t_, den, filler))
                norm_finish(osbs, den, t_)
                while state["i"] < len(units):
                    units[state["i"]]()
                    state["i"] += 1

            for dt_ in range(8):
                proj_unit(2, dt_)

    nc.compile()
    return nc


_NC_CACHE = None


def _get_nc():
    global _NC_CACHE
    if _NC_CACHE is None:
        _NC_CACHE = build_program()
    return _NC_CACHE


def make_in_maps(x, W_qkv, b_qkv, W_out):
    """Per-core input dicts (core c -> batch c//4, head group c%4)."""
    k_ = np.arange(128)[:, None]
    jr = np.arange(128)[None, :]
    umd = (k_ <= jr).astype(BF)                              # [k, jr]
    ic512 = np.arange(512)[None, :]
    wmd = np.where(k_ > ic512, NEG, 0.0).astype(BF)          # [k, ic]
    seld = np.zeros((128, 256), BF)
    seld[0, 0:64] = 1
    seld[32, 64:128] = 1
    seld[64, 128:192] = 1
    seld[96, 192:256] = 1

    in_maps = []
    for c in range(N_CORES):
        b, g = divmod(c, 4)
        rs = slice(DL * g, DL * g + DL)
        wq = W_qkv[0 * D:1 * D][rs]
        wk = W_qkv[1 * D:2 * D][rs]
        wvl = W_qkv[2 * D:3 * D][rs]
        in_maps.append({
            "xt": np.ascontiguousarray(x[b].T).astype(BF),
            "wqk": np.ascontiguousarray(
                np.concatenate([wq, wk], 0).T).astype(BF),
            "wv": np.ascontiguousarray(wvl.T).astype(BF),
            "wout": np.ascontiguousarray(W_out[:, rs].T).astype(BF),
            "bqk": np.ascontiguousarray(
                np.concatenate([b_qkv[0 * D:1 * D][rs],
                                b_qkv[1 * D:2 * D][rs]])[:, None], np.float32),
            "bv": np.ascontiguousarray(b_qkv[2 * D:3 * D][rs][None, :],
                                       np.float32),
            "umd": umd,
            "wmd": wmd,
            "seld": seld,
        })
    return in_maps


def assemble_output(results, b_out):
    y = np.zeros((B, L, D), np.float32)
    for c in range(N_CORES):
        b = c // 4
        y[b] += results[c]["yt"].T.astype(np.float32)
    y += b_out[None, None, :].astype(np.float32)
    return y


def run(x, mask, W_qkv, b_qkv, W_out, b_out, trace=False, **spmd_kwargs):
    causal = np.array_equal(
        np.asarray(mask).reshape(L, L),
        np.triu(np.ones((L, L), bool), k=1))
    if not causal:
        # Fallback (never expected): reference semantics on host.
        print("WARNING: non-causal mask; computing on host")
        q, k, v = np.split(x @ W_qkv.T + b_qkv, 3, axis=-1)
        th = lambda t: t.reshape(B, L, H, HD).transpose(0, 2, 1, 3)
        q, k, v = th(q), th(k), th(v)
        a = np.einsum('bhqd,bhkd->bhqk', q, k) * SCALE
        a = np.where(np.asarray(mask), -np.inf, a)
        a = a - a.max(-1, keepdims=True)
        a = np.exp(a)
        a /= a.sum(-1, keepdims=True)
        o = np.einsum('bhqk,bhkd->bhqd', a, v)
        o = o.transpose(0, 2, 1, 3).reshape(B, L, D)
        return o @ W_out.T + b_out, None

    nc = _get_nc()
    in_maps = make_in_maps(np.asarray(x), np.asarray(W_qkv),
                           np.asarray(b_qkv), np.asarray(W_out))
    res = run_bass_kernel_spmd(nc, in_maps, list(range(N_CORES)),
                               trace=trace, **spmd_kwargs)
    y = assemble_output(res.results, np.asarray(b_out))
    return y, res


def kernel(x, mask, W_qkv, b_qkv, W_out, b_out):
    y, _ = run(x, mask, W_qkv, b_qkv, W_out, b_out)
    return y


# revision 11
# speedup vs baseline: 1.1829x; 1.1829x over previous
"""Causal multi-head attention block (B=2, L=2048, D=1024, H=16) on 8 trn2 cores.

Sharding: core c -> batch b = c // 4, head group g = c % 4 (heads 4g..4g+4).
All matmul operands bf16 (fp32 PSUM accumulation); rel-err gate is 2e-2.

Per core:
  1. QT/KT = (W_qk x^T + b)      (d_head on partitions; 512 x 2048)
  2. V     = (x W_v^T + b_v)     (token j on partitions; [128, 256] per j-chunk)
  3. per (query-block t_, head-pair hp), J = 0..4(t_+1)-1:
     - ST for both heads into one [128,1024] PSUM tile (row-tiled: head A uses
       PE rows 0-63, head B rows 64-127 -> concurrent), causal-narrowed on the
       diagonal; PSUM groups there opened by a bf16 mask matmul
       (sum_k u[k,jr] w[k,ic] = -1e30*max(0, jr-ic)).
     - one exp (no-max softmax, scores ~N(0,1)) -> bf16 P tile
     - PV col-tiled: head A -> PSUM partitions 0-63, head B -> 64-127
       (concurrent); denominators via 2-way col-tiled ones-matmuls into a
       shared per-t_ PSUM tile at partitions 0/32/64/96.
  4. per t_: one reciprocal (PSUM den -> bf16), sel-matmul broadcast,
     elementwise scale -> bf16 ot_t; y^T_partial = W_out,local ot_t -> bf16 DMA
Host: y[b] = sum of the 4 partials^T + b_out.
QKV / out-projection matmuls are interleaved between attention units to keep
the PE busy (HAM stays at full clock).
"""

import numpy as np
import ml_dtypes

import concourse.bass as bass
import concourse.bacc as bacc
import concourse.mybir as mybir
from concourse.tile import TileContext
from concourse.bass_utils import run_bass_kernel_spmd

B, L, D, H = 2, 2048, 1024, 16
HD = 64                      # head dim
HPC = 4                      # heads per core
DL = HPC * HD                # 256 local head dims
N_CORES = 8
NEG = -1.0e30
SCALE = 1.0 / 8.0            # 1/sqrt(64)
FP32 = mybir.dt.float32
BF16 = mybir.dt.bfloat16
AF = mybir.ActivationFunctionType
BF = ml_dtypes.bfloat16

NKC = D // 128               # 8 contraction chunks over D
NMB = L // 512               # 4 column blocks of 512 over L
NJC = L // 128               # 16 j-chunks of 128


def build_program():
    nc = bacc.Bacc("TRN2", target_bir_lowering=False, debug=False)

    xt = nc.dram_tensor("xt", [D, L], BF16, kind="ExternalInput")
    wqk = nc.dram_tensor("wqk", [D, 2 * DL], BF16, kind="ExternalInput")
    wv = nc.dram_tensor("wv", [D, DL], BF16, kind="ExternalInput")
    wout = nc.dram_tensor("wout", [DL, D], BF16, kind="ExternalInput")
    bqk = nc.dram_tensor("bqk", [2 * DL, 1], FP32, kind="ExternalInput")
    bv = nc.dram_tensor("bv", [1, DL], FP32, kind="ExternalInput")
    umd = nc.dram_tensor("umd", [128, 128], BF16, kind="ExternalInput")
    wmd = nc.dram_tensor("wmd", [128, 512], BF16, kind="ExternalInput")
    seld = nc.dram_tensor("seld", [128, 256], BF16, kind="ExternalInput")
    yt = nc.dram_tensor("yt", [D, L], BF16, kind="ExternalOutput")

    with TileContext(nc) as tc:
        with (
            tc.tile_pool(name="const", bufs=1) as const,
            tc.tile_pool(name="xtp", bufs=32) as xtp,
            tc.tile_pool(name="ptp", bufs=3) as ptp,
            tc.tile_pool(name="rp", bufs=2) as rp,
            tc.tile_pool(name="yp", bufs=2) as yp,
            tc.tile_pool(name="ps_st", bufs=1, space="PSUM") as ps_st,
            tc.tile_pool(name="ps_ot", bufs=1, space="PSUM") as ps_ot,
            tc.tile_pool(name="ps_den", bufs=1, space="PSUM") as ps_den,
            tc.tile_pool(name="ps_sm", bufs=2, space="PSUM") as ps_sm,
        ):
            # ---- persistent constants / weights ----
            # Three parallel DMA queues: sync (SP-HWDGE) carries wqk + all of
            # x; scalar (ACT-HWDGE) carries the other weights; gpsimd carries
            # the small biases. The first qk-unit needs wqk + x block 0, so
            # those interleave at the head of the sync queue.
            wqk_t = []
            xts = {m: [] for m in range(NMB)}
            for kc in range(NKC):
                t = const.tile([128, 2 * DL], BF16, tag=f"wqk{kc}")
                nc.sync.dma_start(out=t[:], in_=wqk[kc * 128:(kc + 1) * 128, :])
                wqk_t.append(t)
                tx = xtp.tile([128, 512], BF16, name="t")
                nc.sync.dma_start(
                    out=tx[:], in_=xt[kc * 128:(kc + 1) * 128, 0:512])
                xts[0].append(tx)
            bq_t = []
            for nt in range(4):
                t = const.tile([128, 1], FP32, tag=f"bqk{nt}")
                nc.gpsimd.dma_start(out=t[:],
                                    in_=bqk[nt * 128:(nt + 1) * 128, :])
                bq_t.append(t)

            def load_consts2():
                wv_t = []
                for kc in range(NKC):
                    t = const.tile([128, DL], BF16, tag=f"wv{kc}")
                    nc.scalar.dma_start(out=t[:],
                                        in_=wv[kc * 128:(kc + 1) * 128, :])
                    wv_t.append(t)
                bvrep = const.tile([128, DL], FP32, tag="bvrep")
                nc.scalar.dma_start(out=bvrep[:],
                                    in_=bv[0:1, :].to_broadcast((128, DL)))
                um_t = const.tile([128, 128], BF16, tag="um")
                nc.scalar.dma_start(out=um_t[:], in_=umd[:, :])
                wm_t = const.tile([128, 512], BF16, tag="wm")
                nc.scalar.dma_start(out=wm_t[:], in_=wmd[:, :])
                sel_t = const.tile([128, 256], BF16, tag="sel")
                nc.scalar.dma_start(out=sel_t[:], in_=seld[:, :])
                return wv_t, bvrep, um_t, wm_t, sel_t

            def load_consts3():
                wout_t = []
                for n2 in range(2):
                    t = const.tile([128, D], BF16, tag=f"wout{n2}")
                    nc.scalar.dma_start(out=t[:],
                                        in_=wout[n2 * 128:(n2 + 1) * 128, :])
                    wout_t.append(t)
                return wout_t

            # persistent activations
            # qk_t[0..1]: QT tiles (128 rows: heads {2i,2i+1}); qk_t[2..3]: KT
            qk_t = [const.tile([128, L], BF16, tag=f"qk{nt}", name=f"qk{nt}")
                    for nt in range(4)]
            # V tiles per j-chunk: [128, 256]; head h cols h*64..h*64+64
            v_t = [const.tile([128, DL], BF16, tag=f"v{j}", name=f"v{j}")
                   for j in range(NJC)]
            ot_t = [const.tile([128, L], BF16, tag=f"ot{n2}", name=f"ot{n2}")
                    for n2 in range(2)]

            def load_x(m):
                for kc in range(NKC):
                    t = xtp.tile([128, 512], BF16, name="t")
                    nc.sync.dma_start(
                        out=t[:],
                        in_=xt[kc * 128:(kc + 1) * 128, m * 512:(m + 1) * 512])
                    xts[m].append(t)

            def qk_unit(m, nt):
                ps = ps_sm.tile([128, 512], FP32, tag="ps_sm")
                for kc in range(NKC):
                    nc.tensor.matmul(
                        ps[:],
                        wqk_t[kc][:, nt * 128:(nt + 1) * 128],
                        xts[m][kc][:],
                        start=(kc == 0), stop=(kc == NKC - 1))
                with nc.allow_low_precision(reason="bf16 activations"):
                    nc.vector.tensor_scalar_add(
                        qk_t[nt][:, m * 512:(m + 1) * 512], ps[:], bq_t[nt][:])

            def v_unit(m, ic):
                j = 4 * m + ic
                ps = ps_sm.tile([128, 512], FP32, tag="ps_sm")
                for kc in range(NKC):
                    nc.tensor.matmul(
                        ps[:, 0:DL],
                        xts[m][kc][:, ic * 128:(ic + 1) * 128],
                        wv_t[kc][:],
                        start=(kc == 0), stop=(kc == NKC - 1))
                with nc.allow_low_precision(reason="bf16 activations"):
                    nc.vector.tensor_add(v_t[j][:], ps[:, 0:DL], bvrep[:])

            def attn_pair(hp, t_, den, filler):
                """One head pair's ST -> exp -> PV/den chain over all j-chunks,
                software-pipelined and processed two j-chunks per step inside
                one [128,2048] PSUM tile so off-diagonal exps fuse into a
                single [128,2048] ACTIVATE (diagonal: one strided ACTIVATE per
                chunk). ST row-tiled (heads at PE rows 0-63/64-127), PV
                col-tiled (output partitions 0-63/64-127) -> concurrent."""
                n_j = 4 * (t_ + 1)
                n_off = 4 * t_          # off-diagonal chunks (multiple of 4)
                qt = qk_t[hp]
                kt = qk_t[2 + hp]
                otp = ps_ot.tile([128, 512], FP32, tag="ps_ot",
                                 name=f"otp{hp}")
                big = ps_st.tile([128, 2048], FP32, tag="ps_st", name="big")

                def do_st(J):
                    q = J - 4 * t_      # >= 0 on the diagonal band
                    w0 = 128 * q if q > 0 else 0
                    base = (J % 2) * 1024
                    for i in range(2):
                        po = i * 64
                        ssl = slice(base + i * 512 + w0, base + (i + 1) * 512)
                        if q >= 0:
                            # open the psum group with the causal mask:
                            # sum_k u[k,jr] w[k,ic] = -1e30*max(0, jr-ic)
                            nc.tensor.matmul(
                                big[:, ssl],
                                um_t[:], wm_t[:, 0:512 - w0],
                                start=True, stop=False)
                        nc.tensor.matmul(
                            big[:, ssl],
                            kt[po:po + 64, J * 128:(J + 1) * 128],
                            qt[po:po + 64, t_ * 512 + w0:(t_ + 1) * 512],
                            start=(q < 0), stop=True)

                def do_exp(J):
                    """exp for chunk pair (J, J+1) -> one [128,2048] P tile."""
                    ptile = ptp.tile([128, 2048], BF16, name="pt")
                    if J >= n_off:
                        for k in (J, J + 1):
                            w0 = 128 * (k - 4 * t_) if k > 4 * t_ else 0
                            base = (k % 2) * 1024
                            src = big[:, base:base + 1024].rearrange(
                                "p (c n) -> p c n", n=512)[:, :, w0:512]
                            dst = ptile[:, base:base + 1024].rearrange(
                                "p (c n) -> p c n", n=512)[:, :, w0:512]
                            nc.scalar.activation(dst, src, AF.Exp, scale=SCALE)
                    else:
                        nc.scalar.activation(ptile[:], big[:],
                                             AF.Exp, scale=SCALE)
                    return ptile

                def do_pv(J, ptile):
                    q = J - 4 * t_
                    w0 = 128 * q if q > 0 else 0
                    base = (J % 2) * 1024
                    for i in range(2):
                        h = 2 * hp + i
                        nc.tensor.matmul(
                            otp[i * 64:(i + 1) * 64, w0:512],
                            v_t[J][:, (h % 4) * 64:(h % 4) * 64 + 64],
                            ptile[:, base + i * 512 + w0:base + (i + 1) * 512],
                            start=(J == 0), stop=(J == n_j - 1),
                            skip_group_check=True)
                    for i in range(2):
                        dr = 32 * (2 * hp + i)
                        nc.tensor.matmul(
                            den[dr:dr + 1, w0:512],
                            um_t[:, 127:128],
                            ptile[:, base + i * 512 + w0:base + (i + 1) * 512],
                            start=(J == 0), stop=(J == n_j - 1),
                            tile_position=(0, dr),
                            skip_group_check=True)

                do_st(0)
                do_st(1)
                for J in range(0, n_j, 2):
                    ptile = do_exp(J)
                    if J + 2 < n_j:
                        do_st(J + 2)
                        do_st(J + 3)
                    filler()
                    do_pv(J, ptile)
                    do_pv(J + 1, ptile)
                    filler()
                # copy O^T|pair out of PSUM (releases the PV psum slot)
                osb = rp.tile([128, 512], FP32, name="osb", tag=f"osb{hp}")
                nc.vector.tensor_copy(osb[:], otp[:])
                return osb

            def norm_finish(osbs, den, t_):
                """One reciprocal for all 4 heads' denominators, broadcast via
                sel-matmul, scale into ot_t."""
                isl = slice(t_ * 512, (t_ + 1) * 512)
                linv = rp.tile([128, 512], BF16, name="linv", tag="linv")
                with nc.allow_low_precision(reason="bf16 norm scale"):
                    nc.vector.reciprocal(linv[:], den[:])
                for n2 in range(2):
                    rb = ps_sm.tile([128, 512], FP32, tag="ps_sm")
                    nc.tensor.matmul(rb[:],
                                     sel_t[:, n2 * 128:(n2 + 1) * 128],
                                     linv[:], start=True, stop=True)
                    with nc.allow_low_precision(reason="bf16 activations"):
                        nc.vector.tensor_mul(ot_t[n2][:, isl], osbs[n2][:],
                                             rb[:])

            def proj_unit(t_, dt_):
                isl = slice(t_ * 512, (t_ + 1) * 512)
                ps = ps_sm.tile([128, 512], FP32, tag="ps_sm")
                for n2 in range(2):
                    nc.tensor.matmul(
                        ps[:],
                        wout_t[n2][:, dt_ * 128:(dt_ + 1) * 128],
                        ot_t[n2][:, isl],
                        start=(n2 == 0), stop=(n2 == 1))
                ys = yp.tile([128, 512], BF16, name="ys")
                with nc.allow_low_precision(reason="bf16 output"):
                    nc.vector.tensor_copy(ys[:], ps[:])
                nc.sync.dma_start(
                    out=yt[dt_ * 128:(dt_ + 1) * 128, isl], in_=ys[:])

            # ---- program ----
            wv_t, bvrep, um_t, wm_t, sel_t = load_consts2()
            wout_t = load_consts3()
            for m in (1, 2, 3):
                load_x(m)
            # only what t_=0 pair 0 needs up front: QT/KT heads 0-1 + first
            # two V chunks; the rest of block 0 becomes t_=0 filler work
            qk_unit(0, 0)
            qk_unit(0, 2)
            v_unit(0, 0)
            v_unit(0, 1)

            # attention block order (0, 1, 3, 2): t_=0 starts right after
            # QKV block 0; its fillers compute QKV blocks 1-2; t_=1 gets
            # block 3 + proj(0); t_=3 gets proj(1); t_=2 gets proj(3);
            # proj(2) trails at the end.
            filler_plan = {0: [], 1: [], 3: [], 2: []}
            filler_plan[0].append(lambda: v_unit(0, 2))
            filler_plan[0].append(lambda: v_unit(0, 3))
            filler_plan[0].append(lambda: qk_unit(0, 1))
            filler_plan[0].append(lambda: qk_unit(0, 3))
            for m in (1, 2):
                for u in range(4):
                    filler_plan[0].append(lambda u=u, m=m: qk_unit(m, u))
                    filler_plan[0].append(lambda u=u, m=m: v_unit(m, u))
            for u in range(4):
                filler_plan[1].append(lambda u=u: qk_unit(3, u))
                filler_plan[1].append(lambda u=u: v_unit(3, u))
            for dt_ in range(8):
                filler_plan[1].append(lambda dt_=dt_: proj_unit(0, dt_))
                filler_plan[3].append(lambda dt_=dt_: proj_unit(1, dt_))
                filler_plan[2].append(lambda dt_=dt_: proj_unit(3, dt_))

            for t_ in (0, 1, 3, 2):
                units = filler_plan[t_]
                n_slots = 2 * 4 * (t_ + 1)   # filler call sites this block
                state = {"i": 0, "slot": 0}

                def filler(state=state, units=units, n_slots=n_slots):
                    # spread the unit supply evenly over the block's slots
                    state["slot"] += 1
                    want = (len(units) * state["slot"] + n_slots - 1) // n_slots
                    while state["i"] < min(want, len(units)):
                        units[state["i"]]()
                        state["i"] += 1

                den = ps_den.tile([128, 512], FP32, tag="ps_den",
                                  name=f"den{t_}")
                nc.vector.memset(den[:], 1.0)
                osbs = []
                for hp in range(2):
                    osbs.append(attn_pair(hp, t_, den, filler))
                norm_finish(osbs, den, t_)
                while state["i"] < len(units):
                    units[state["i"]]()
                    state["i"] += 1

            for dt_ in range(8):
                proj_unit(2, dt_)

    nc.compile()
    return nc


_NC_CACHE = None


def _get_nc():
    global _NC_CACHE
    if _NC_CACHE is None:
        _NC_CACHE = build_program()
    return _NC_CACHE


def make_in_maps(x, W_qkv, b_qkv, W_out):
    """Per-core input dicts (core c -> batch c//4, head group c%4)."""
    k_ = np.arange(128)[:, None]
    jr = np.arange(128)[None, :]
    umd = (k_ <= jr).astype(BF)                              # [k, jr]
    ic512 = np.arange(512)[None, :]
    wmd = np.where(k_ > ic512, NEG, 0.0).astype(BF)          # [k, ic]
    seld = np.zeros((128, 256), BF)
    seld[0, 0:64] = 1
    seld[32, 64:128] = 1
    seld[64, 128:192] = 1
    seld[96, 192:256] = 1

    in_maps = []
    for c in range(N_CORES):
        b, g = divmod(c, 4)
        rs = slice(DL * g, DL * g + DL)
        wq = W_qkv[0 * D:1 * D][rs]
        wk = W_qkv[1 * D:2 * D][rs]
        wvl = W_qkv[2 * D:3 * D][rs]
        in_maps.append({
            "xt": np.ascontiguousarray(x[b].T).astype(BF),
            "wqk": np.ascontiguousarray(
                np.concatenate([wq, wk], 0).T).astype(BF),
            "wv": np.ascontiguousarray(wvl.T).astype(BF),
            "wout": np.ascontiguousarray(W_out[:, rs].T).astype(BF),
            "bqk": np.ascontiguousarray(
                np.concatenate([b_qkv[0 * D:1 * D][rs],
                                b_qkv[1 * D:2 * D][rs]])[:, None], np.float32),
            "bv": np.ascontiguousarray(b_qkv[2 * D:3 * D][rs][None, :],
                                       np.float32),
            "umd": umd,
            "wmd": wmd,
            "seld": seld,
        })
    return in_maps


def assemble_output(results, b_out):
    y = np.zeros((B, L, D), np.float32)
    for c in range(N_CORES):
        b = c // 4
        y[b] += results[c]["yt"].T.astype(np.float32)
    y += b_out[None, None, :].astype(np.float32)
    return y


def run(x, mask, W_qkv, b_qkv, W_out, b_out, trace=False, **spmd_kwargs):
    causal = np.array_equal(
        np.asarray(mask).reshape(L, L),
        np.triu(np.ones((L, L), bool), k=1))
    if not causal:
        # Fallback (never expected): reference semantics on host.
        print("WARNING: non-causal mask; computing on host")
        q, k, v = np.split(x @ W_qkv.T + b_qkv, 3, axis=-1)
        th = lambda t: t.reshape(B, L, H, HD).transpose(0, 2, 1, 3)
        q, k, v = th(q), th(k), th(v)
        a = np.einsum('bhqd,bhkd->bhqk', q, k) * SCALE
        a = np.where(np.asarray(mask), -np.inf, a)
        a = a - a.max(-1, keepdims=True)
        a = np.exp(a)
        a /= a.sum(-1, keepdims=True)
        o = np.einsum('bhqk,bhkd->bhqd', a, v)
        o = o.transpose(0, 2, 1, 3).reshape(B, L, D)
        return o @ W_out.T + b_out, None

    nc = _get_nc()
    in_maps = make_in_maps(np.asarray(x), np.asarray(W_qkv),
                           np.asarray(b_qkv), np.asarray(W_out))
    res = run_bass_kernel_spmd(nc, in_maps, list(range(N_CORES)),
                               trace=trace, **spmd_kwargs)
    y = assemble_output(res.results, np.asarray(b_out))
    return y, res


def kernel(x, mask, W_qkv, b_qkv, W_out, b_out):
    y, _ = run(x, mask, W_qkv, b_qkv, W_out, b_out)
    return y


# revision 15
# speedup vs baseline: 1.2265x; 1.0369x over previous
"""Causal multi-head attention block (B=2, L=2048, D=1024, H=16) on 8 trn2 cores.

Sharding: core c -> batch b = c // 4, head group g = c % 4 (heads 4g..4g+4).
All matmul operands bf16 (fp32 PSUM accumulation); rel-err gate is 2e-2.

Per core:
  1. QT/KT = (W_qk x^T + b)      (d_head on partitions; 512 x 2048)
  2. V     = (x W_v^T + b_v)     (token j on partitions; [128, 256] per j-chunk)
  3. per (query-block t_, head-pair hp), J = 0..4(t_+1)-1:
     - ST for both heads into one [128,1024] PSUM tile (row-tiled: head A uses
       PE rows 0-63, head B rows 64-127 -> concurrent), causal-narrowed on the
       diagonal; PSUM groups there opened by a bf16 mask matmul
       (sum_k u[k,jr] w[k,ic] = -1e30*max(0, jr-ic)).
     - one exp (no-max softmax, scores ~N(0,1)) -> bf16 P tile
     - PV col-tiled: head A -> PSUM partitions 0-63, head B -> 64-127
       (concurrent); denominators via 2-way col-tiled ones-matmuls into a
       shared per-t_ PSUM tile at partitions 0/32/64/96.
  4. per t_: one reciprocal (PSUM den -> bf16), sel-matmul broadcast,
     elementwise scale -> bf16 ot_t; y^T_partial = W_out,local ot_t -> bf16 DMA
Host: y[b] = sum of the 4 partials^T + b_out.
QKV / out-projection matmuls are interleaved between attention units to keep
the PE busy (HAM stays at full clock).
"""

import numpy as np
import ml_dtypes

import concourse.bass as bass
import concourse.bacc as bacc
import concourse.mybir as mybir
from concourse.tile import TileContext
from concourse.bass_utils import run_bass_kernel_spmd

B, L, D, H = 2, 2048, 1024, 16
HD = 64                      # head dim
HPC = 4                      # heads per core
DL = HPC * HD                # 256 local head dims
N_CORES = 8
NEG = -1.0e30
SCALE = 1.0 / 8.0            # 1/sqrt(64)
FP32 = mybir.dt.float32
BF16 = mybir.dt.bfloat16
AF = mybir.ActivationFunctionType
BF = ml_dtypes.bfloat16

NKC = D // 128               # 8 contraction chunks over D
NMB = L // 512               # 4 column blocks of 512 over L
NJC = L // 128               # 16 j-chunks of 128


def build_program():
    nc = bacc.Bacc("TRN2", target_bir_lowering=False, debug=False)

    xt = nc.dram_tensor("xt", [D, L], BF16, kind="ExternalInput")
    wqk = nc.dram_tensor("wqk", [D, 2 * DL], BF16, kind="ExternalInput")
    wv = nc.dram_tensor("wv", [D, DL], BF16, kind="ExternalInput")
    wout = nc.dram_tensor("wout", [DL, D], BF16, kind="ExternalInput")
    bqk = nc.dram_tensor("bqk", [2 * DL, 1], FP32, kind="ExternalInput")
    bv = nc.dram_tensor("bv", [1, DL], FP32, kind="ExternalInput")
    umd = nc.dram_tensor("umd", [128, 128], BF16, kind="ExternalInput")
    wmd = nc.dram_tensor("wmd", [128, 512], BF16, kind="ExternalInput")
    seld = nc.dram_tensor("seld", [128, 256], BF16, kind="ExternalInput")
    yt = nc.dram_tensor("yt", [D, L], BF16, kind="ExternalOutput")

    with TileContext(nc) as tc:
        with (
            tc.tile_pool(name="const", bufs=1) as const,
            tc.tile_pool(name="xtp", bufs=32) as xtp,
            tc.tile_pool(name="ptp", bufs=3) as ptp,
            tc.tile_pool(name="rp", bufs=2) as rp,
            tc.tile_pool(name="yp", bufs=2) as yp,
            tc.tile_pool(name="ps_st", bufs=1, space="PSUM") as ps_st,
            tc.tile_pool(name="ps_ot", bufs=1, space="PSUM") as ps_ot,
            tc.tile_pool(name="ps_den", bufs=1, space="PSUM") as ps_den,
            tc.tile_pool(name="ps_sm", bufs=2, space="PSUM") as ps_sm,
        ):
            # ---- persistent constants / weights ----
            # Three parallel DMA queues: sync (SP-HWDGE) carries wqk + all of
            # x; scalar (ACT-HWDGE) carries the other weights; gpsimd carries
            # the small biases. The first qk-unit needs wqk + x block 0, so
            # those interleave at the head of the sync queue.
            wqk_t = []
            xts = {m: [] for m in range(NMB)}
            for kc in range(NKC):
                t = const.tile([128, 2 * DL], BF16, tag=f"wqk{kc}")
                nc.scalar.dma_start(out=t[:],
                                    in_=wqk[kc * 128:(kc + 1) * 128, :])
                wqk_t.append(t)
                tx = xtp.tile([128, 512], BF16, name="t")
                nc.sync.dma_start(
                    out=tx[:], in_=xt[kc * 128:(kc + 1) * 128, 0:512])
                xts[0].append(tx)
            bq_t = []
            for nt in range(4):
                t = const.tile([128, 1], FP32, tag=f"bqk{nt}")
                nc.gpsimd.dma_start(out=t[:],
                                    in_=bqk[nt * 128:(nt + 1) * 128, :])
                bq_t.append(t)

            def load_consts2():
                wv_t = []
                for kc in range(NKC):
                    t = const.tile([128, DL], BF16, tag=f"wv{kc}")
                    nc.scalar.dma_start(out=t[:],
                                        in_=wv[kc * 128:(kc + 1) * 128, :])
                    wv_t.append(t)
                bvrep = const.tile([128, DL], FP32, tag="bvrep")
                nc.gpsimd.dma_start(out=bvrep[:],
                                    in_=bv[0:1, :].to_broadcast((128, DL)))
                um_t = const.tile([128, 128], BF16, tag="um")
                nc.gpsimd.dma_start(out=um_t[:], in_=umd[:, :])
                wm_t = const.tile([128, 512], BF16, tag="wm")
                nc.gpsimd.dma_start(out=wm_t[:], in_=wmd[:, :])
                sel_t = const.tile([128, 256], BF16, tag="sel")
                nc.gpsimd.dma_start(out=sel_t[:], in_=seld[:, :])
                return wv_t, bvrep, um_t, wm_t, sel_t

            def load_consts3():
                wout_t = []
                for n2 in range(2):
                    t = const.tile([128, D], BF16, tag=f"wout{n2}")
                    nc.scalar.dma_start(out=t[:],
                                        in_=wout[n2 * 128:(n2 + 1) * 128, :])
                    wout_t.append(t)
                return wout_t

            # persistent activations
            # qk_t[0..1]: QT tiles (128 rows: heads {2i,2i+1}); qk_t[2..3]: KT
            qk_t = [const.tile([128, L], BF16, tag=f"qk{nt}", name=f"qk{nt}")
                    for nt in range(4)]
            # V tiles per j-chunk: [128, 256]; head h cols h*64..h*64+64
            v_t = [const.tile([128, DL], BF16, tag=f"v{j}", name=f"v{j}")
                   for j in range(NJC)]
            ot_t = [const.tile([128, L], BF16, tag=f"ot{n2}", name=f"ot{n2}")
                    for n2 in range(2)]

            def load_x(m):
                for kc in range(NKC):
                    t = xtp.tile([128, 512], BF16, name="t")
                    nc.sync.dma_start(
                        out=t[:],
                        in_=xt[kc * 128:(kc + 1) * 128, m * 512:(m + 1) * 512])
                    xts[m].append(t)

            def qk_unit(m, nt):
                ps = ps_sm.tile([128, 512], FP32, tag="ps_sm")
                for kc in range(NKC):
                    nc.tensor.matmul(
                        ps[:],
                        wqk_t[kc][:, nt * 128:(nt + 1) * 128],
                        xts[m][kc][:],
                        start=(kc == 0), stop=(kc == NKC - 1))
                with nc.allow_low_precision(reason="bf16 activations"):
                    nc.vector.tensor_scalar_add(
                        qk_t[nt][:, m * 512:(m + 1) * 512], ps[:], bq_t[nt][:])

            def v_unit(m, ic):
                j = 4 * m + ic
                ps = ps_sm.tile([128, 512], FP32, tag="ps_sm")
                for kc in range(NKC):
                    nc.tensor.matmul(
                        ps[:, 0:DL],
                        xts[m][kc][:, ic * 128:(ic + 1) * 128],
                        wv_t[kc][:],
                        start=(kc == 0), stop=(kc == NKC - 1))
                with nc.allow_low_precision(reason="bf16 activations"):
                    nc.vector.tensor_add(v_t[j][:], ps[:, 0:DL], bvrep[:])

            def attn_pair(hp, t_, den, filler):
                """One head pair's ST -> exp -> PV/den chain over all j-chunks,
                software-pipelined and processed two j-chunks per step inside
                one [128,2048] PSUM tile so off-diagonal exps fuse into a
                single [128,2048] ACTIVATE (diagonal: one strided ACTIVATE per
                chunk). ST row-tiled (heads at PE rows 0-63/64-127), PV
                col-tiled (output partitions 0-63/64-127) -> concurrent."""
                n_j = 4 * (t_ + 1)
                n_off = 4 * t_          # off-diagonal chunks (multiple of 4)
                qt = qk_t[hp]
                kt = qk_t[2 + hp]
                otp = ps_ot.tile([128, 512], FP32, tag="ps_ot",
                                 name=f"otp{hp}")
                big = ps_st.tile([128, 2048], FP32, tag="ps_st", name="big")

                def do_st(J):
                    q = J - 4 * t_      # >= 0 on the diagonal band
                    w0 = 128 * q if q > 0 else 0
                    base = (J % 2) * 1024
                    for i in range(2):
                        po = i * 64
                        ssl = slice(base + i * 512 + w0, base + (i + 1) * 512)
                        if q >= 0:
                            # open the psum group with the causal mask:
                            # sum_k u[k,jr] w[k,ic] = -1e30*max(0, jr-ic)
                            nc.tensor.matmul(
                                big[:, ssl],
                                um_t[:], wm_t[:, 0:512 - w0],
                                start=True, stop=False)
                        nc.tensor.matmul(
                            big[:, ssl],
                            kt[po:po + 64, J * 128:(J + 1) * 128],
                            qt[po:po + 64, t_ * 512 + w0:(t_ + 1) * 512],
                            start=(q < 0), stop=True)

                def do_exp(J):
                    """exp for chunk pair (J, J+1) -> one [128,2048] P tile."""
                    ptile = ptp.tile([128, 2048], BF16, name="pt")
                    if J >= n_off:
                        for k in (J, J + 1):
                            w0 = 128 * (k - 4 * t_) if k > 4 * t_ else 0
                            base = (k % 2) * 1024
                            src = big[:, base:base + 1024].rearrange(
                                "p (c n) -> p c n", n=512)[:, :, w0:512]
                            dst = ptile[:, base:base + 1024].rearrange(
                                "p (c n) -> p c n", n=512)[:, :, w0:512]
                            nc.scalar.activation(dst, src, AF.Exp, scale=SCALE)
                    else:
                        nc.scalar.activation(ptile[:], big[:],
                                             AF.Exp, scale=SCALE)
                    return ptile

                def do_pv(J, ptile):
                    q = J - 4 * t_
                    w0 = 128 * q if q > 0 else 0
                    base = (J % 2) * 1024
                    for i in range(2):
                        h = 2 * hp + i
                        nc.tensor.matmul(
                            otp[i * 64:(i + 1) * 64, w0:512],
                            v_t[J][:, (h % 4) * 64:(h % 4) * 64 + 64],
                            ptile[:, base + i * 512 + w0:base + (i + 1) * 512],
                            start=(J == 0), stop=(J == n_j - 1),
                            skip_group_check=True)
                    for i in range(2):
                        dr = 32 * (2 * hp + i)
                        nc.tensor.matmul(
                            den[dr:dr + 1, w0:512],
                            um_t[:, 127:128],
                            ptile[:, base + i * 512 + w0:base + (i + 1) * 512],
                            start=(J == 0), stop=(J == n_j - 1),
                            tile_position=(0, dr),
                            skip_group_check=True)

                do_st(0)
                do_st(1)
                for J in range(0, n_j, 2):
                    ptile = do_exp(J)
                    if J + 2 < n_j:
                        do_st(J + 2)
                        do_st(J + 3)
                    filler()
                    do_pv(J, ptile)
                    do_pv(J + 1, ptile)
                    filler()
                # copy O^T|pair out of PSUM (releases the PV psum slot)
                osb = rp.tile([128, 512], FP32, name="osb", tag=f"osb{hp}")
                nc.vector.tensor_copy(osb[:], otp[:])
                return osb

            def norm_finish(osbs, den, t_):
                """One reciprocal for all 4 heads' denominators, broadcast via
                sel-matmul, scale into ot_t."""
                isl = slice(t_ * 512, (t_ + 1) * 512)
                linv = rp.tile([128, 512], BF16, name="linv", tag="linv")
                with nc.allow_low_precision(reason="bf16 norm scale"):
                    nc.vector.reciprocal(linv[:], den[:])
                for n2 in range(2):
                    rb = ps_sm.tile([128, 512], FP32, tag="ps_sm")
                    nc.tensor.matmul(rb[:],
                                     sel_t[:, n2 * 128:(n2 + 1) * 128],
                                     linv[:], start=True, stop=True)
                    with nc.allow_low_precision(reason="bf16 activations"):
                        nc.vector.tensor_mul(ot_t[n2][:, isl], osbs[n2][:],
                                             rb[:])

            def proj_unit(t_, dt_, tail=False):
                isl = slice(t_ * 512, (t_ + 1) * 512)
                ps = ps_sm.tile([128, 512], FP32, tag="ps_sm")
                for n2 in range(2):
                    nc.tensor.matmul(
                        ps[:],
                        wout_t[n2][:, dt_ * 128:(dt_ + 1) * 128],
                        ot_t[n2][:, isl],
                        start=(n2 == 0), stop=(n2 == 1))
                ys = yp.tile([128, 512], BF16, name="ys")
                with nc.allow_low_precision(reason="bf16 output"):
                    if tail:
                        # the scalar engine is idle at the kernel tail
                        nc.scalar.copy(ys[:], ps[:])
                    else:
                        nc.vector.tensor_copy(ys[:], ps[:])
                nc.sync.dma_start(
                    out=yt[dt_ * 128:(dt_ + 1) * 128, isl], in_=ys[:])

            # ---- program ----
            wv_t, bvrep, um_t, wm_t, sel_t = load_consts2()
            wout_t = load_consts3()
            for m in (1, 2, 3):
                load_x(m)
            # only what t_=0 pair 0 needs up front: QT/KT heads 0-1 + first
            # two V chunks; the rest of block 0 becomes t_=0 filler work
            qk_unit(0, 0)
            qk_unit(0, 2)
            v_unit(0, 0)
            v_unit(0, 1)

            # attention block order (0, 1, 3, 2): t_=0 starts right after
            # QKV block 0; its fillers compute QKV blocks 1-2; t_=1 gets
            # block 3 + proj(0); t_=3 gets proj(1); t_=2 gets proj(3);
            # proj(2) trails at the end.
            filler_plan = {0: [], 1: [], 3: [], 2: []}
            filler_plan[0].append(lambda: v_unit(0, 2))
            filler_plan[0].append(lambda: v_unit(0, 3))
            filler_plan[0].append(lambda: qk_unit(0, 1))
            filler_plan[0].append(lambda: qk_unit(0, 3))
            for u in range(4):
                filler_plan[0].append(lambda u=u: qk_unit(1, u))
                filler_plan[0].append(lambda u=u: v_unit(1, u))
            for m in (2, 3):
                for u in range(4):
                    filler_plan[1].append(lambda u=u, m=m: qk_unit(m, u))
                    filler_plan[1].append(lambda u=u, m=m: v_unit(m, u))
            for dt_ in range(8):
                filler_plan[3].append(lambda dt_=dt_: proj_unit(0, dt_))
                filler_plan[3].append(lambda dt_=dt_: proj_unit(1, dt_))
                filler_plan[2].append(lambda dt_=dt_: proj_unit(3, dt_))

            for t_ in (0, 1, 3, 2):
                units = filler_plan[t_]
                n_slots = 2 * 4 * (t_ + 1)   # filler call sites this block
                state = {"i": 0, "slot": 0}

                def filler(state=state, units=units, n_slots=n_slots):
                    # spread the unit supply evenly over the block's slots
                    state["slot"] += 1
                    want = (len(units) * state["slot"] + n_slots - 1) // n_slots
                    while state["i"] < min(want, len(units)):
                        units[state["i"]]()
                        state["i"] += 1

                den = ps_den.tile([128, 512], FP32, tag="ps_den",
                                  name=f"den{t_}")
                nc.vector.memset(den[:], 1.0)
                osbs = []
                for hp in range(2):
                    osbs.append(attn_pair(hp, t_, den, filler))
                norm_finish(osbs, den, t_)
                while state["i"] < len(units):
                    units[state["i"]]()
                    state["i"] += 1

            for dt_ in range(8):
                proj_unit(2, dt_, tail=True)

    nc.compile()
    return nc


_NC_CACHE = None


def _get_nc():
    global _NC_CACHE
    if _NC_CACHE is None:
        _NC_CACHE = build_program()
    return _NC_CACHE


def make_in_maps(x, W_qkv, b_qkv, W_out):
    """Per-core input dicts (core c -> batch c//4, head group c%4)."""
    k_ = np.arange(128)[:, None]
    jr = np.arange(128)[None, :]
    umd = (k_ <= jr).astype(BF)                              # [k, jr]
    ic512 = np.arange(512)[None, :]
    wmd = np.where(k_ > ic512, NEG, 0.0).astype(BF)          # [k, ic]
    seld = np.zeros((128, 256), BF)
    seld[0, 0:64] = 1
    seld[32, 64:128] = 1
    seld[64, 128:192] = 1
    seld[96, 192:256] = 1

    in_maps = []
    for c in range(N_CORES):
        b, g = divmod(c, 4)
        rs = slice(DL * g, DL * g + DL)
        wq = W_qkv[0 * D:1 * D][rs]
        wk = W_qkv[1 * D:2 * D][rs]
        wvl = W_qkv[2 * D:3 * D][rs]
        in_maps.append({
            "xt": np.ascontiguousarray(x[b].T).astype(BF),
            "wqk": np.ascontiguousarray(
                np.concatenate([wq, wk], 0).T).astype(BF),
            "wv": np.ascontiguousarray(wvl.T).astype(BF),
            "wout": np.ascontiguousarray(W_out[:, rs].T).astype(BF),
            "bqk": np.ascontiguousarray(
                np.concatenate([b_qkv[0 * D:1 * D][rs],
                                b_qkv[1 * D:2 * D][rs]])[:, None], np.float32),
            "bv": np.ascontiguousarray(b_qkv[2 * D:3 * D][rs][None, :],
                                       np.float32),
            "umd": umd,
            "wmd": wmd,
            "seld": seld,
        })
    return in_maps


def assemble_output(results, b_out):
    y = np.zeros((B, L, D), np.float32)
    for c in range(N_CORES):
        b = c // 4
        y[b] += results[c]["yt"].T.astype(np.float32)
    y += b_out[None, None, :].astype(np.float32)
    return y


def run(x, mask, W_qkv, b_qkv, W_out, b_out, trace=False, **spmd_kwargs):
    causal = np.array_equal(
        np.asarray(mask).reshape(L, L),
        np.triu(np.ones((L, L), bool), k=1))
    if not causal:
        # Fallback (never expected): reference semantics on host.
        print("WARNING: non-causal mask; computing on host")
        q, k, v = np.split(x @ W_qkv.T + b_qkv, 3, axis=-1)
        th = lambda t: t.reshape(B, L, H, HD).transpose(0, 2, 1, 3)
        q, k, v = th(q), th(k), th(v)
        a = np.einsum('bhqd,bhkd->bhqk', q, k) * SCALE
        a = np.where(np.asarray(mask), -np.inf, a)
        a = a - a.max(-1, keepdims=True)
        a = np.exp(a)
        a /= a.sum(-1, keepdims=True)
        o = np.einsum('bhqk,bhkd->bhqd', a, v)
        o = o.transpose(0, 2, 1, 3).reshape(B, L, D)
        return o @ W_out.T + b_out, None

    nc = _get_nc()
    in_maps = make_in_maps(np.asarray(x), np.asarray(W_qkv),
                           np.asarray(b_qkv), np.asarray(W_out))
    res = run_bass_kernel_spmd(nc, in_maps, list(range(N_CORES)),
                               trace=trace, **spmd_kwargs)
    y = assemble_output(res.results, np.asarray(b_out))
    return y, res


def kernel(x, mask, W_qkv, b_qkv, W_out, b_out):
    y, _ = run(x, mask, W_qkv, b_qkv, W_out, b_out)
    return y
